# revision 87
# baseline (speedup 1.0000x reference)
import sys
import numpy as np

sys.path.insert(0, "/opt/trn_rl_repo")

N_CORES = 8
B_FULL, C, H, W = 64, 128, 80, 80
BL = B_FULL // N_CORES  # 8 batches per core
HW = H * W  # 6400
NH, HD = 4, 32
SIG2 = 2.0 * 0.3**2
G = 4  # batches per group
XSLOTS = 8
NCH = 50  # 128-wide hw chunks per batch
SC = [(k * 1024, 1024) for k in range(6)] + [(6144, 256)]  # super-chunks

_CACHE = {}


def _build():
    import concourse.bass as bass
    import concourse.tile as tile
    from concourse import bacc, mybir

    f32 = mybir.dt.float32
    f32r = mybir.dt.float32r
    bf16 = mybir.dt.bfloat16
    AF = mybir.ActivationFunctionType
    OP = mybir.AluOpType
    AX = mybir.AxisListType

    nc = bacc.Bacc("TRN2", target_bir_lowering=False, debug=False)
    d = {}
    d["x"] = nc.dram_tensor("x", [BL, C, H, W], f32, kind="ExternalInput").ap()
    d["masks"] = nc.dram_tensor("masks", [BL, 2, 640, 640], f32, kind="ExternalInput").ap()
    F32_CONSTS = [
        ("wqt", [128, 128]), ("wkt", [128, 128]), ("wvt", [128, 128]), ("wot", [128, 128]),
        ("w1t", [128, 256]), ("w2t0", [128, 128]), ("w2t1", [128, 128]),
        ("bq", [128, 1]), ("bk", [128, 1]), ("bv", [128, 1]), ("bo", [128, 1]),
        ("b1", [128, 2]), ("b2", [128, 1]),
        ("g1", [128, 1]), ("be1", [128, 1]), ("g2", [128, 1]), ("be2", [128, 1]),
        ("idf", [128, 128]), ("hself", [128, 4]),
        ("xs2", [1, 160]), ("ys80", [80, 1]), ("sgn8", [1, 2 * G]),
        ("ygmap", [128, 100]), ("xgmap", [128, 100]),
    ]
    F32R_CONSTS = [
        ("ones128r", [1, 128]), ("ones14r", [1, 4]), ("ones128c", [128, 1]),
        ("hselr", [4, 128]), ("nselpair", [2, 128]),
        ("ones80b", [80, 1]), ("idb", [128, 128]),
    ]
    for nm, shp in F32_CONSTS + F32R_CONSTS:
        d[nm] = nc.dram_tensor(nm, shp, f32, kind="ExternalInput").ap()
    out_d = nc.dram_tensor("out", [BL, C, H, W], bf16, kind="ExternalOutput").ap()

    with tile.TileContext(nc) as tc:
        from contextlib import ExitStack

        ctx = ExitStack()
        cpool = ctx.enter_context(tc.tile_pool(name="consts", bufs=1))
        xpool = ctx.enter_context(tc.tile_pool(name="x", bufs=1))       # X tags
        ypool = ctx.enter_context(tc.tile_pool(name="yout", bufs=2))
        mpool = ctx.enter_context(tc.tile_pool(name="mask", bufs=2))    # Mf
        spool = ctx.enter_context(tc.tile_pool(name="stat", bufs=2))    # sstat, tx
        stpool = ctx.enter_context(tc.tile_pool(name="stp", bufs=5))    # ST ring
        bpool = ctx.enter_context(tc.tile_pool(name="brow", bufs=2))    # Brow
        xtpool = ctx.enter_context(tc.tile_pool(name="xtsb", bufs=4))   # XTsb ring
        mrpool = ctx.enter_context(tc.tile_pool(name="mr2", bufs=2))    # mr2, mT
        tiny = ctx.enter_context(tc.tile_pool(name="tiny", bufs=2))
        ph3 = ctx.enter_context(tc.tile_pool(name="ph3", bufs=2))
        gpool = ctx.enter_context(tc.tile_pool(name="grp", bufs=2))     # group tiles
        ps_y = ctx.enter_context(tc.tile_pool(name="psy", bufs=3, space="PSUM"))   # [128,512] f32 x3 = 3 banks
        ps_xt = ctx.enter_context(tc.tile_pool(name="psxt", bufs=3, space="PSUM"))  # [128,1024] bf16 x3 = 3 banks
        ps_t = ctx.enter_context(tc.tile_pool(name="pst", bufs=1, space="PSUM"))    # tiny matmuls, 1 bank
        ps_nr = ctx.enter_context(tc.tile_pool(name="psnr", bufs=1, space="PSUM"))  # NRp, 1 bank

        cst = {}
        for nm, _ in F32_CONSTS:
            t = cpool.tile(list(d[nm].shape), f32, tag=nm)
            nc.scalar.dma_start(t[:], d[nm][:])
            cst[nm] = t
        EARLY_R = ("ones80b", "idb")
        def load_r_consts(names):
            for nm, _ in F32R_CONSTS:
                if nm not in names:
                    continue
                dt_ = bf16 if nm in ("ones80b", "idb") else f32r
                t = cpool.tile(list(d[nm].shape), dt_, tag=nm)
                nc.gpsimd.dma_start(t[:], d[nm][:])
                cst[nm] = t
        load_r_consts(EARLY_R)

        def new_group(g):
            NR = gpool.tile([128, 2 * G], f32, tag="NR")
            S6 = gpool.tile([1, 6 * G], f32, tag="S6")
            CX = gpool.tile([1, 2 * G], f32, tag="CX")
            NRp = ps_nr.tile([128, 2 * G], f32, tag="NRp")
            return {"g": g, "Xb": {}, "mT": {}, "NR": NR, "S6": S6, "CX": CX,
                    "NRp": NRp}

        def S1stat(gr, i):
            b = G * gr["g"] + i
            S6, CX = gr["S6"], gr["CX"]
            Mf = mpool.tile([80, 1280], bf16, tag="Mf")
            src = d["masks"][b].rearrange("n (h s) w -> h n s w", s=8)[:, :, 0, :]
            nc.gpsimd.dma_start(Mf[:].rearrange("h (n w) -> h n w", n=2), src)

            # ST = [msub(160,(n,w)) | mbin(160) | mby(160)]
            ST = stpool.tile([80, 480], bf16, tag="ST")
            for n in range(2):
                sub = Mf[:, n * 640:(n + 1) * 640].rearrange("h (w s) -> h w s", s=8)[:, :, 0]
                nc.vector.tensor_copy(out=ST[:, n * 80:(n + 1) * 80], in_=sub)
            nc.vector.tensor_scalar(out=ST[:, 160:320], in0=ST[:, 0:160], scalar1=0.5,
                                    scalar2=None, op0=OP.is_gt)
            nc.vector.tensor_scalar(out=ST[:, 320:480], in0=ST[:, 160:320],
                                    scalar1=cst["ys80"][:], scalar2=None, op0=OP.mult)

            pstat = ps_t.tile([1, 480], f32, tag="pst")
            nc.tensor.matmul(pstat[:], cst["ones80b"][:], ST[:], start=True, stop=True)
            sstat = spool.tile([1, 480], f32, tag="sstat")
            nc.scalar.activation(out=sstat[:], in_=pstat[:], func=AF.Copy)
            nc.vector.tensor_reduce(out=S6[:, 6 * i:6 * i + 6],
                                    in_=sstat[:].rearrange("p (k w) -> p k w", w=80),
                                    axis=AX.X, op=OP.add)
            tx = spool.tile([1, 160], f32, tag="tx")
            nc.vector.tensor_tensor(out=tx[:], in0=sstat[:, 160:320], in1=cst["xs2"][:], op=OP.mult)
            nc.vector.tensor_reduce(out=CX[:, 2 * i:2 * i + 2],
                                    in_=tx[:].rearrange("p (k w) -> p k w", w=80),
                                    axis=AX.X, op=OP.add)
            gr.setdefault("ST", {})[i] = ST

        def S1loadX(gr, i):
            b = G * gr["g"] + i
            X = xpool.tile([128, HW], bf16, tag=f"X{b % XSLOTS}")
            gr["Xb"][i] = X
            xsrc = d["x"][b].rearrange("c h w -> c (h w)")
            nc.gpsimd.dma_start(X[:, 0:2048], xsrc[:, 0:2048])
            nc.gpsimd.dma_start(X[:, 2048:4096], xsrc[:, 2048:4096])
            nc.gpsimd.dma_start(X[:, 4096:HW], xsrc[:, 4096:HW])

        def S1loadM(gr, i):
            """Mask-transpose (mT) prep for batch i of group gr."""
            ST = gr["ST"][i]
            mrf = mrpool.tile([2, HW], bf16, tag="mrf", bufs=1)
            mr2 = mrpool.tile([50, 256], bf16, tag="mr2", bufs=3)
            for n in range(2):
                nc.sync.dma_start(mrf[n:n + 1, :], ST[:, n * 80:(n + 1) * 80])
                nc.sync.dma_start(mr2[:, n * 128:(n + 1) * 128], mrf[n:n + 1, :])
            mTp = ps_xt.tile([128, 1024], bf16, tag="XTp")
            for n in range(2):
                nc.tensor.transpose(mTp[:, n * 50:(n + 1) * 50],
                                    mr2[:, n * 128:(n + 1) * 128],
                                    cst["idb"][0:50, 0:50])
            mT = mrpool.tile([128, 100], bf16, tag="mT", bufs=4)
            nc.vector.tensor_copy(out=mT[:], in_=mTp[:, 0:100])
            gr["mT"][i] = mT

        def pool_units(gr, i, engines):
            """Callables emitting pooling work for batch i of group gr,
            stepped per super-chunk: (xtrans_c, copy_c, poolmm_{c-1}).
            engines[c] picks the PSUM->SBUF copy engine per super-chunk."""
            X = gr["Xb"][i]
            mT = gr["mT"][i]
            NRp = gr["NRp"]
            mTv = mT[:].rearrange("p (n j) -> p n j", n=2)
            state = {}

            def xtrans(c):
                off, ln = SC[c]
                nch = ln // 128
                XTp = ps_xt.tile([128, 1024], bf16, tag="XTp")
                state[c] = XTp
                for kk in range(nch):
                    nc.tensor.transpose(XTp[:, kk * 128:(kk + 1) * 128],
                                        X[:, off + kk * 128:off + (kk + 1) * 128],
                                        cst["idb"][:])

            def xtcopy(c):
                off, ln = SC[c]
                XTs = xtpool.tile([128, 1024], bf16, tag="XTsb")
                eng = engines[c]
                if eng == "act":
                    nc.scalar.activation(out=XTs[:, 0:ln], in_=state[c][:, 0:ln], func=AF.Copy)
                else:
                    nc.vector.tensor_copy(out=XTs[:, 0:ln], in_=state[c][:, 0:ln])
                state[c] = XTs

            def poolmm(c):
                off, ln = SC[c]
                nch = ln // 128
                XTs = state.pop(c)
                for kk in range(nch):
                    j = 8 * c + kk
                    nc.tensor.matmul(NRp[:, 2 * i:2 * i + 2],
                                     XTs[:, kk * 128:(kk + 1) * 128],
                                     mTv[:, :, j],
                                     start=(j == 0), stop=(j == NCH - 1),
                                     skip_group_check=True)

            def finish():
                nc.vector.tensor_copy(out=gr["NR"][:, 2 * i:2 * i + 2],
                                      in_=NRp[:, 2 * i:2 * i + 2])

            return xtrans, xtcopy, poolmm, finish

        PRO_ENG = ["dve", "act", "dve", "act", "dve", "act", "dve"]
        STEADY_ENG = ["act", "dve", "act", "dve", "act", "dve", "act"]

        def S1pool(gr, i):
            """Standalone pooling (prologue + group tails), shift-1 pipelined."""
            xtrans, xtcopy, poolmm, finish = pool_units(gr, i, PRO_ENG)
            for c in range(len(SC)):
                xtrans(c)
                xtcopy(c)
                if c >= 1:
                    poolmm(c - 1)
            poolmm(len(SC) - 1)
            finish()

        def layernorm(res0, res1, gname, bname, tag, out_dtype):
            st8 = gpool.tile([128, 4 * G], f32r, tag=tag + "s")
            nc.vector.tensor_tensor(out=st8[:, 0:2 * G], in0=res0, in1=res1, op=OP.add)
            nc.scalar.activation(out=st8[:, 2 * G:4 * G], in_=st8[:, 0:2 * G], func=AF.Square)
            pl = ps_t.tile([1, 4 * G], f32, tag="pst")
            nc.tensor.matmul(pl[:], cst["ones128c"][:], st8[:], start=True, stop=True)
            ms = gpool.tile([1, 4 * G], f32r, tag=tag + "m")  # mean 0:8 | rstd 8:16
            nc.vector.tensor_scalar(out=ms[:, 0:2 * G], in0=pl[:, 0:2 * G], scalar1=1.0 / 128,
                                    scalar2=None, op0=OP.mult)
            vtmp = gpool.tile([1, 2 * G], f32, tag=tag + "v")
            nc.vector.tensor_tensor(out=vtmp[:], in0=ms[:, 0:2 * G], in1=ms[:, 0:2 * G], op=OP.mult)
            nc.vector.tensor_scalar(out=ms[:, 2 * G:4 * G], in0=pl[:, 2 * G:4 * G], scalar1=1.0 / 128,
                                    scalar2=1e-5, op0=OP.mult, op1=OP.add)
            nc.vector.tensor_tensor(out=ms[:, 2 * G:4 * G], in0=ms[:, 2 * G:4 * G], in1=vtmp[:], op=OP.subtract)
            # rstd = sqrt(1/var)
            with nc.allow_low_precision(reason="f32r stores full fp32 bits"):
                nc.vector.reciprocal(out=ms[:, 2 * G:4 * G], in_=ms[:, 2 * G:4 * G])
            nc.scalar.activation(out=ms[:, 2 * G:4 * G], in_=ms[:, 2 * G:4 * G], func=AF.Sqrt)
            pms = ps_t.tile([128, 4 * G], f32, tag="pst")
            nc.tensor.matmul(pms[:], cst["ones128r"][:], ms[:], start=True, stop=True)
            hh = gpool.tile([128, 2 * G], out_dtype, tag=tag + "h")
            nc.vector.tensor_tensor(out=hh[:], in0=st8[:, 0:2 * G], in1=pms[:, 0:2 * G], op=OP.subtract)
            nc.vector.tensor_tensor(out=hh[:], in0=hh[:], in1=pms[:, 2 * G:4 * G], op=OP.mult)
            nc.vector.tensor_scalar(out=hh[:], in0=hh[:], scalar1=cst[gname][:],
                                    scalar2=cst[bname][:], op0=OP.mult, op1=OP.add)
            return hh

        def S2a(gr):
            S6, CX = gr["S6"], gr["CX"]
            S6v = S6[:].rearrange("p (b k) -> p b k", k=6)
            rmsum = gpool.tile([1, 2 * G], f32r, tag="rmsum")
            nc.vector.tensor_scalar(out=rmsum[:].rearrange("p (b k) -> p b k", k=2),
                                    in0=S6v[:, :, 0:2], scalar1=1e-6, scalar2=None, op0=OP.add)
            with nc.allow_low_precision(reason="f32r stores full fp32 bits"):
                nc.vector.reciprocal(out=rmsum[:], in_=rmsum[:])
            rcnt = gpool.tile([1, 2 * G], f32, tag="rcnt")
            nc.vector.tensor_copy(out=rcnt[:].rearrange("p (b k) -> p b k", k=2), in_=S6v[:, :, 2:4])
            nc.vector.reciprocal(out=rcnt[:], in_=rcnt[:])
            PXPY = gpool.tile([1, 4 * G], f32r, tag="PXPY")
            cxy = gpool.tile([1, 4 * G], f32, tag="cxy")  # cx | cy
            nc.vector.tensor_tensor(out=cxy[:, 0:2 * G], in0=CX[:], in1=rcnt[:], op=OP.mult)
            nc.vector.tensor_tensor(out=cxy[:, 2 * G:4 * G].rearrange("p (b k) -> p b k", k=2),
                                    in0=S6v[:, :, 4:6], in1=rcnt[:].rearrange("p (b k) -> p b k", k=2),
                                    op=OP.mult)
            nc.vector.tensor_scalar(out=PXPY[:], in0=cxy[:], scalar1=2.0 / 80.0, scalar2=-1.0,
                                    op0=OP.mult, op1=OP.add)
            dxy = gpool.tile([1, 2 * G], f32, tag="dxy")
            PXv = PXPY[:].rearrange("p (xy b n) -> p xy b n", xy=2, n=2)
            nc.vector.tensor_tensor(out=dxy[:].rearrange("p (xy b) -> p xy b", xy=2),
                                    in0=PXv[:, :, :, 0], in1=PXv[:, :, :, 1], op=OP.subtract)
            nc.vector.tensor_tensor(out=dxy[:], in0=dxy[:], in1=dxy[:], op=OP.mult)
            d4 = gpool.tile([1, G], f32, tag="d4")
            nc.vector.scalar_tensor_tensor(out=d4[:], in0=dxy[:, 0:G], scalar=1e-12,
                                           in1=dxy[:, G:2 * G], op0=OP.add, op1=OP.add)
            nc.scalar.activation(out=d4[:], in_=d4[:], func=AF.Sqrt)
            dpm = gpool.tile([1, 2 * G], f32r, tag="dpm")
            dbc = d4[:].rearrange("p (b u) -> p b u", u=1).broadcast_to((1, G, 2))
            nc.vector.tensor_tensor(out=dpm[:].rearrange("p (b n) -> p b n", n=2),
                                    in0=dbc, in1=cst["sgn8"][:].rearrange("p (b n) -> p b n", n=2),
                                    op=OP.mult)

            pdm = ps_t.tile([4, 2 * G], f32, tag="pst")
            nc.tensor.matmul(pdm[:], cst["ones14r"][:], dpm[:], start=True, stop=True)
            dpm_sb = gpool.tile([4, 2 * G], f32, tag="dpm_sb")
            nc.scalar.activation(out=dpm_sb[:], in_=pdm[:], func=AF.Copy)
            PXYp = gpool.tile([2, 2 * G], f32r, tag="PXYp")
            for n in range(2):
                nc.sync.dma_start(PXYp[n:n + 1, 0:G],
                                  PXPY[:, 0:2 * G].rearrange("p (b n) -> p b n", n=2)[:, :, n])
                nc.sync.dma_start(PXYp[n:n + 1, G:2 * G],
                                  PXPY[:, 2 * G:4 * G].rearrange("p (b n) -> p b n", n=2)[:, :, n])
            pxy_ps = ps_t.tile([128, 2 * G], f32, tag="pst")
            nc.tensor.matmul(pxy_ps[:], cst["nselpair"][:], PXYp[:], start=True, stop=True)
            pxys = gpool.tile([128, 2 * G], f32, tag="pxys")
            nc.scalar.activation(out=pxys[:], in_=pxy_ps[:], func=AF.Copy)
            gr["pxys"] = pxys
            gr["rmsum"] = rmsum
            gr["dpm_sb"] = dpm_sb

        def S2b(gr):
            NR = gr["NR"]
            rmsum = gr["rmsum"]
            pmul = ps_t.tile([128, 2 * G], f32, tag="pst")
            nc.tensor.matmul(pmul[:], cst["ones128r"][:], rmsum[:], start=True, stop=True)
            nodes = gpool.tile([128, 2 * G], f32, tag="nodes")
            nc.vector.tensor_tensor(out=nodes[:], in0=NR[:], in1=pmul[:], op=OP.mult)

            qkv = gpool.tile([128, 6 * G], f32, tag="qkv")
            for k, (wt, bb) in enumerate([("wqt", "bq"), ("wkt", "bk"), ("wvt", "bv")]):
                pq = ps_t.tile([128, 2 * G], f32, tag="pst")
                nc.tensor.matmul(pq[:], cst[wt][:], nodes[:], start=True, stop=True)
                nc.scalar.activation(out=qkv[:, 2 * G * k:2 * G * (k + 1)], in_=pq[:], func=AF.Identity,
                                     bias=cst[bb][:])
            P = gpool.tile([128, 4 * G], f32, tag="P")
            qv = qkv[:, 0:2 * G].rearrange("p (b n u) -> p b n u", n=2, u=1).broadcast_to((128, G, 2, 2))
            kv = qkv[:, 2 * G:4 * G].rearrange("p (b m u) -> p b u m", m=2, u=1).broadcast_to((128, G, 2, 2))
            nc.vector.tensor_tensor(out=P[:].rearrange("p (b n m) -> p b n m", n=2, m=2),
                                    in0=qv, in1=kv, op=OP.mult)
            pscore = ps_t.tile([4, 4 * G], f32, tag="pst")
            nc.tensor.matmul(pscore[:], cst["hself"][:], P[:], start=True, stop=True)
            sc_sb = gpool.tile([4, 4 * G], f32, tag="sc_sb")
            nc.scalar.activation(out=sc_sb[:], in_=pscore[:], func=AF.Copy)
            sdiff = gpool.tile([4, 2 * G], f32, tag="sdiff")
            scv = sc_sb[:].rearrange("p (b n m) -> p b n m", n=2, m=2)
            nc.vector.tensor_tensor(out=sdiff[:].rearrange("p (b n) -> p b n", n=2),
                                    in0=scv[:, :, :, 0], in1=scv[:, :, :, 1], op=OP.subtract)
            z8 = gpool.tile([4, 2 * G], f32, tag="z8")
            nc.vector.scalar_tensor_tensor(out=z8[:], in0=sdiff[:], scalar=1.0 / np.sqrt(32.0),
                                           in1=gr["dpm_sb"][:], op0=OP.mult, op1=OP.add)
            a4r = gpool.tile([4, 2 * G], f32r, tag="a4r")
            nc.scalar.activation(out=a4r[:], in_=z8[:], func=AF.Sigmoid)
            pa = ps_t.tile([128, 2 * G], f32, tag="pst")
            nc.tensor.matmul(pa[:], cst["hselr"][:], a4r[:], start=True, stop=True)
            vv = qkv[:, 4 * G:6 * G].rearrange("p (b n) -> p b n", n=2)
            vd = gpool.tile([128, G], f32, tag="vd")
            nc.vector.tensor_tensor(out=vd[:], in0=vv[:, :, 0], in1=vv[:, :, 1], op=OP.subtract)
            ao = gpool.tile([128, 2 * G], f32, tag="ao")
            vdb = vd[:].rearrange("p (b u) -> p b u", u=1).broadcast_to((128, G, 2))
            v1b = vv[:, :, 1:2].broadcast_to((128, G, 2))
            nc.vector.tensor_tensor(out=ao[:].rearrange("p (b n) -> p b n", n=2),
                                    in0=pa[:].rearrange("p (b n) -> p b n", n=2), in1=vdb, op=OP.mult)
            nc.vector.tensor_tensor(out=ao[:].rearrange("p (b n) -> p b n", n=2),
                                    in0=ao[:].rearrange("p (b n) -> p b n", n=2), in1=v1b, op=OP.add)

            po = ps_t.tile([128, 2 * G], f32, tag="pst")
            nc.tensor.matmul(po[:], cst["wot"][:], ao[:], start=True, stop=True)
            y1 = gpool.tile([128, 2 * G], f32, tag="y1")
            nc.scalar.activation(out=y1[:], in_=po[:], func=AF.Identity, bias=cst["bo"][:])
            h1 = layernorm(y1[:], nodes[:], "g1", "be1", "ln1", f32)

            zz = gpool.tile([128, 4 * G], f32, tag="zz")
            for k in range(2):
                pz = ps_t.tile([128, 2 * G], f32, tag="pst")
                nc.tensor.matmul(pz[:], cst["w1t"][:, 128 * k:128 * k + 128], h1[:],
                                 start=True, stop=True)
                nc.scalar.activation(out=zz[:, 2 * G * k:2 * G * (k + 1)], in_=pz[:], func=AF.Relu,
                                     bias=cst["b1"][:, k:k + 1])
            py2 = ps_t.tile([128, 2 * G], f32, tag="pst")
            nc.tensor.matmul(py2[:], cst["w2t0"][:], zz[:, 0:2 * G], start=True, stop=False)
            nc.tensor.matmul(py2[:], cst["w2t1"][:], zz[:, 2 * G:4 * G], start=False, stop=True)
            y2 = gpool.tile([128, 2 * G], f32, tag="y2")
            nc.scalar.activation(out=y2[:], in_=py2[:], func=AF.Identity, bias=cst["b2"][:])
            h2 = layernorm(y2[:], h1[:], "g2", "be2", "ln2", f32)

            gr["h2T"] = []
            for i in range(G):
                ptr = ps_t.tile([2, 128], f32, tag="pst")
                nc.tensor.transpose(ptr[:], h2[:, 2 * i:2 * i + 2], cst["idf"][:])
                hT = ph3.tile([2, 128], bf16, tag=f"h2T{i}")
                nc.scalar.activation(out=hT[:], in_=ptr[:], func=AF.Copy)
                gr["h2T"].append(hT)

        def bbuild(gr, i):
            """Gaussian splat row-pair (Brow) for batch i of group gr."""
            pxys = gr["pxys"]
            s1 = ph3.tile([128, 100], f32, tag="bb1")
            s2 = ph3.tile([128, 100], f32, tag="bb2")
            nc.gpsimd.tensor_scalar(out=s1[:], in0=cst["ygmap"][:], scalar1=pxys[:, G + i:G + i + 1],
                                    scalar2=None, op0=OP.subtract)
            nc.gpsimd.tensor_scalar(out=s2[:], in0=cst["xgmap"][:], scalar1=pxys[:, i:i + 1],
                                    scalar2=None, op0=OP.subtract)
            nc.gpsimd.tensor_tensor(out=s1[:], in0=s1[:], in1=s1[:], op=OP.mult)
            nc.gpsimd.tensor_tensor(out=s2[:], in0=s2[:], in1=s2[:], op=OP.mult)
            nc.gpsimd.tensor_tensor(out=s1[:], in0=s1[:], in1=s2[:], op=OP.add)
            Bi = ph3.tile([128, 100], bf16, tag="Bt")
            nc.scalar.activation(out=Bi[:], in_=s1[:], func=AF.Exp, scale=-1.0 / SIG2)
            Brow = bpool.tile([2, HW], bf16, tag="Brow")
            for n in range(2):
                nc.sync.dma_start(Brow[n:n + 1, :], Bi[64 * n:64 * n + 64, :])
            gr.setdefault("Brow", {})[i] = Brow

        CH512 = [(k * 512, 512) for k in range(12)] + [(6144, 256)]
        # Y-add engine per 512-chunk: act chunks use PE idb-matmul + Act copy
        YP_POOL = ["dve", "act", "dve", "act", "dve", "dve", "act",
                   "dve", "act", "dve", "act", "dve", "dve"]
        YP_PLAIN = ["dve", "act", "dve", "act", "dve", "act", "dve",
                    "act", "dve", "act", "dve", "act", "dve"]

        def S13(cur, i, pool_list):
            """Splat batch i of group cur; interleave pooling for the
            pool_list [(group, batch), ...] batches."""
            b = G * cur["g"] + i
            if i + 1 < G:
                bbuild(cur, i + 1)
            X = cur["Xb"][i]
            Brow = cur["Brow"][i]
            hT = cur["h2T"][i]
            Y = ypool.tile([128, HW], bf16, tag="Y", bufs=2)
            units = [pool_units(g_, i_, STEADY_ENG if k == 0 else STEADY_ENG[1:] + STEADY_ENG[:1])
                     for k, (g_, i_) in enumerate(pool_list)]
            ypat = YP_POOL if units else YP_PLAIN
            for c in range(len(CH512)):
                off, ln = CH512[c]
                if c < len(SC):
                    for xtrans, xtcopy, poolmm, finish in units:
                        xtrans(c)
                Yp = ps_y.tile([128, 512], f32, tag="Yp")
                eng = ypat[c]
                if eng == "act":
                    nc.tensor.matmul(Yp[:, 0:ln], hT[:], Brow[:, off:off + ln],
                                     start=True, stop=False, skip_group_check=True)
                    nc.tensor.matmul(Yp[:, 0:ln], cst["idb"][:], X[:, off:off + ln],
                                     start=False, stop=True, skip_group_check=True)
                else:
                    nc.tensor.matmul(Yp[:, 0:ln], hT[:], Brow[:, off:off + ln],
                                     start=True, stop=True, skip_group_check=True)
                if c < len(SC):
                    for xtrans, xtcopy, poolmm, finish in units:
                        xtcopy(c)
                        if c >= 2:
                            poolmm(c - 2)
                if eng == "act":
                    nc.scalar.activation(out=Y[:, off:off + ln], in_=Yp[:, 0:ln], func=AF.Copy)
                else:
                    nc.vector.tensor_tensor(out=Y[:, off:off + ln], in0=Yp[:, 0:ln],
                                            in1=X[:, off:off + ln], op=OP.add)
                if c == 2:
                    yout = out_d[b].rearrange("c h w -> c (h w)")
                    nc.gpsimd.dma_start(yout[:, 0:1536], Y[:, 0:1536])
                elif c == 5:
                    nc.gpsimd.dma_start(yout[:, 1536:3072], Y[:, 1536:3072])
                elif c == 8:
                    nc.gpsimd.dma_start(yout[:, 3072:4608], Y[:, 3072:4608])
            for xtrans, xtcopy, poolmm, finish in units:
                poolmm(len(SC) - 2)
                poolmm(len(SC) - 1)
                finish()
            nc.gpsimd.dma_start(yout[:, 4608:HW], Y[:, 4608:HW])

        NGROUP = BL // G
        gr = new_group(0)
        load_r_consts([nm for nm, _ in F32R_CONSTS if nm not in EARLY_R])
        S1loadX(gr, 0)
        for i in range(G):
            S1stat(gr, i)
        for i in range(1, G):
            S1loadX(gr, i)
        S2a(gr)
        for i in range(G):
            S1loadM(gr, i)
            S1pool(gr, i)
        S2b(gr)
        bbuild(gr, 0)
        for g in range(NGROUP):
            cur = gr
            nxt = new_group(g + 1) if g + 1 < NGROUP else None
            if nxt is not None:
                for i in range(G):
                    S1stat(nxt, i)
                S2a(nxt)
                S1loadX(nxt, 0)
                S1loadX(nxt, 1)
            for i in range(G):
                plist = []
                if nxt is not None and i >= 1:
                    S1loadM(nxt, i - 1)
                    plist.append((nxt, i - 1))
                if nxt is not None and i == G - 1:
                    S1loadM(nxt, G - 1)
                    S1pool(nxt, G - 1)
                S13(cur, i, plist)
                if nxt is not None and i == 0:
                    S1loadX(nxt, 2)
                    S1loadX(nxt, 3)
            if nxt is not None:
                S2b(nxt)
                bbuild(nxt, 0)
            gr = nxt
        ctx.close()

    nc.compile()
    return nc


def _consts_np(inputs):
    f = np.float32
    wq = np.asarray(inputs["wq"], f); wk = np.asarray(inputs["wk"], f)
    wv = np.asarray(inputs["wv"], f); wo = np.asarray(inputs["wo"], f)
    w1 = np.asarray(inputs["w1"], f); w2 = np.asarray(inputs["w2"], f)

    yg = np.linspace(-1.0, 1.0, 80, dtype=f)
    idx = np.arange(12800)
    rem = idx % 6400
    ygmap = yg[(rem // 80)].reshape(128, 100).astype(f)
    xgmap = yg[(rem % 80)].reshape(128, 100).astype(f)
    hself = np.zeros((128, 4), f)
    for h in range(4):
        hself[h * 32:(h + 1) * 32, h] = 1.0
    nselpair = np.zeros((2, 128), f)
    nselpair[0, 0:64] = 1.0
    nselpair[1, 64:128] = 1.0

    return {
        "wqt": np.ascontiguousarray(wq.T), "wkt": np.ascontiguousarray(wk.T),
        "wvt": np.ascontiguousarray(wv.T), "wot": np.ascontiguousarray(wo.T),
        "w1t": np.ascontiguousarray(w1.T),
        "w2t0": np.ascontiguousarray(w2.T[0:128]), "w2t1": np.ascontiguousarray(w2.T[128:256]),
        "bq": np.asarray(inputs["bq"], f).reshape(128, 1),
        "bk": np.asarray(inputs["bk"], f).reshape(128, 1),
        "bv": np.asarray(inputs["bv"], f).reshape(128, 1),
        "bo": np.asarray(inputs["bo"], f).reshape(128, 1),
        "b1": np.ascontiguousarray(np.asarray(inputs["b1"], f).reshape(2, 128).T),
        "b2": np.asarray(inputs["b2"], f).reshape(128, 1),
        "g1": np.asarray(inputs["ln1_g"], f).reshape(128, 1),
        "be1": np.asarray(inputs["ln1_b"], f).reshape(128, 1),
        "g2": np.asarray(inputs["ln2_g"], f).reshape(128, 1),
        "be2": np.asarray(inputs["ln2_b"], f).reshape(128, 1),
        "idf": np.eye(128, dtype=f),
        "hself": hself, "hselr": np.ascontiguousarray(hself.T),
        "nselpair": nselpair,
        "xs2": np.tile(np.arange(80, dtype=f), 2).reshape(1, 160),
        "ys80": np.arange(80, dtype=f).reshape(80, 1),
        "sgn8": np.tile(np.array([1.0, -1.0], f), G).reshape(1, 2 * G),
        "ygmap": ygmap, "xgmap": xgmap,
        "ones128r": np.ones((1, 128), f),
        "ones128c": np.ones((128, 1), f),
        "ones14r": np.ones((1, 4), f),
        "ones80b": np.ones((80, 1), f),
        "idb": np.eye(128, dtype=f),
    }


def kernel(**inputs):
    from concourse.bass_utils import run_bass_kernel_spmd

    x = np.asarray(inputs["x"], dtype=np.float32)
    masks = np.asarray(inputs["masks"], dtype=np.float32)
    consts = _consts_np(inputs)

    if "nc" not in _CACHE:
        _CACHE["nc"] = _build()
    nc = _CACHE["nc"]

    in_maps = []
    for c in range(N_CORES):
        m = {"x": np.ascontiguousarray(x[c * BL:(c + 1) * BL]),
             "masks": np.ascontiguousarray(masks[c * BL:(c + 1) * BL])}
        m.update(consts)
        in_maps.append(m)
    _CACHE["in_maps"] = in_maps
    res = run_bass_kernel_spmd(nc, in_maps, list(range(N_CORES))).results
    return np.concatenate([np.asarray(r["out"], dtype=np.float32) for r in res], axis=0)
